# revision 30
# baseline (speedup 1.0000x reference)
"""Trainium2 Bass kernel for sparse-attention (full 16384x16384 attention,
64-dim head, 64x64 projections).

Sharding: query rows split across 8 cores (2048 rows each); hidden (K/V
source) and the 64x64 projection weights are replicated on every core.

Per-core algorithm:
  prologue (all-bf16, all matmuls use full-128-row stationaries so the PE
  HAM clock gate sees a busy array and runs at 2.4 GHz):
    warmup   ~16 dummy matmuls gated on the first h16 tile: un-throttle the
             PE HAM clock gate right before the real matmuls start
    h16      hidden loaded f32 (DMAs split across the SP and ACT dispatch
             queues), converted to bf16 (DVE-heavy)
    hT       PE-transposed 128x128 blocks of h16; 16 packed tiles [128,512],
             top half = even kv chunks' hidden^T, bottom = odd chunks'
    kT       k^T via zero-padded stationaries wk2_top=[[Wk],[0]],
             wk2_bot=[[0],[Wk]] (K=128 keeps the HAM warm; the zero half
             multiplies the other parity's data away exactly).  Stored
             parity-blocked (tile s = [even chunks | odd chunks]) so the
             PSUM->SBUF copies stay contiguous.
    qT       q^T likewise [128, 2048] (rows 64:128 exact zeros)
    v        v in natural [kv, c] chunks, each padded with a ones column
             (the ones column makes stage 2 accumulate the softmax
             denominator as row 64 of outT)
  steady state, per q-pass (1024 q cols) x 128 kv chunks, software-
  pipelined so the PE streams gap-free at 2.4 GHz (863 ns/chunk):
    stage1: scA[kv=128,512], scB[kv=128,512] = kT_chunk^T @ qT  (PE->PSUM)
    exp:    expA = exp(scA/8) on ACT; expB = Schraudolph bf16 bit-pattern
            round(A*scB+B) as int16 on DVE.  Two engines in parallel, each
            under the PE's 864 ns/chunk budget; fully separate tiles so the
            tile framework cannot serialize them.
    stage2: outT[65, q] += v_chunk_aug^T @ exp{A,B}            (PE, PSUM)
  epilogue per pass: outT -> SBUF (copy split ACT/DVE), PE-transpose
  128-col blocks, multiply by reciprocal of the ones-column sum
  (alternating DVE/ACT), DMA out.

kv ordering is permuted (chunk c holds kv rows {128p + c}) - harmless since
softmax fully reduces over kv and k/v use the same permutation.  q ordering:
column g = 128t + p corresponds to query row 16p + t of this core's shard.
"""

import numpy as np

import concourse.bass as bass
import concourse.mybir as mybir
import concourse.tile as tile
from concourse import bacc
from concourse.bass_utils import run_bass_kernel_spmd
from concourse.masks import make_identity

F32 = mybir.dt.float32
BF16 = mybir.dt.bfloat16
I16 = mybir.dt.int16
AF = mybir.ActivationFunctionType

# Schraudolph exp for bf16: bitpattern(exp(s/8)) ~= round(A*s + B).
# A = 2^7 * log2(e) / 8 (folds the 1/sqrt(64) score scale); B centers the
# exponent at 127 with c=0.04368 minimizing max relative error (~3%).
SCH_A = 128.0 * 0.125 * 1.4426950408889634
SCH_B = 128.0 * (127.0 - 0.04368)

NKV = 16384
C = 64
NCORES = 8
NQ = NKV // NCORES  # 2048 q rows per core
P = 128

NKVC = NKV // P      # 128 kv chunks
NQC = NQ // P        # 16 q chunks
QPASS = 1024         # q columns processed per pass
NPASS = NQ // QPASS  # 2

import os
NKVC_RUN = int(os.environ.get("NKVC_RUN", NKVC))
NPASS_RUN = int(os.environ.get("NPASS_RUN", NPASS))
SCB = int(os.environ.get("SCB", "2"))
XB = int(os.environ.get("XB", "4"))
WARMN = int(os.environ.get("WARMN", "16"))


def _copy(eng, out, in_):
    if hasattr(eng, "tensor_copy"):
        eng.tensor_copy(out=out, in_=in_)
    else:
        eng.copy(out=out, in_=in_)


def _emit(nc, tc, ctx_, hidden_d, query_d, wq_d, wk_d, wv_d, out_d):
    from contextlib import ExitStack

    # ---------------- constants ----------------
    consts = ctx_.enter_context(tc.tile_pool(name="consts", bufs=1))
    # top/bot variants: w*t = [[W],[0]], w*b = [[0],[W]] so projections from
    # the parity-packed hT tiles contract over all 128 partitions (the zero
    # half exactly cancels the other parity's rows)
    wq2 = [consts.tile([P, P], BF16, name=f"wq2{i}") for i in range(2)]
    wk2 = [consts.tile([P, P], BF16, name=f"wk2{i}") for i in range(2)]
    wv2 = [consts.tile([P, C], BF16, name=f"wv2{i}") for i in range(2)]
    # one staging tile per weight: a shared one serializes DMA->copy->DMA
    wsts = [consts.tile([P, 2 * P], F32, name=f"wst{i}") for i in range(3)]
    ones16 = consts.tile([P, 16], BF16, name="ones16")
    identb = consts.tile([P, P], BF16, name="identb")
    identf = consts.tile([P, P], F32, name="identf")
    warmw = consts.tile([P, P], BF16, name="warmw")
    warmm = consts.tile([P, 512], BF16, name="warmm")

    nc.vector.memset(warmw[:], 0.0)
    nc.vector.memset(warmm[:], 0.0)
    nc.vector.memset(ones16[:], 1.0)
    make_identity(nc, identb)
    make_identity(nc, identf)

    # persistent operand tiles
    big = ctx_.enter_context(tc.tile_pool(name="big", bufs=1))
    kT_tiles = [big.tile([P, 8 * P], BF16, tag=f"kt{i}", name=f"kt{i}")
                for i in range(16)]
    qT = big.tile([P, NQ], BF16, tag="qt", name="qT")
    v_tiles = [big.tile([P, 16 * 65], BF16, tag=f"v{i}", name=f"v{i}")
               for i in range(8)]

    hidden_r = hidden_d.rearrange("(p t) c -> p (t c)", p=P)   # [128, 8192]
    query_r = query_d.rearrange("(p t) c -> p (t c)", p=P)     # [128, 1024]
    out_flat = out_d.rearrange("(p t) c -> p (t c)", p=P)      # [128, 1024]

    with ExitStack() as pro:
        hp = pro.enter_context(tc.tile_pool(name="hnat", bufs=1))
        htp = pro.enter_context(tc.tile_pool(name="ht", bufs=1))
        tpp = pro.enter_context(
            tc.tile_pool(name="tp_psum", bufs=2, space="PSUM"))
        prp = pro.enter_context(
            tc.tile_pool(name="proj_psum", bufs=2, space="PSUM"))
        wpp = pro.enter_context(
            tc.tile_pool(name="warm_psum", bufs=2, space="PSUM"))

        # ---- weight DMAs first: tiny (16KB), and wk2 gates the projection
        # chain; separate stagings avoid DMA->copy->DMA serialization
        for i, w_d in enumerate((wq_d, wk_d, wv_d)):
            nc.vector.memset(wsts[i][:], 0.0)
            nc.sync.dma_start(out=wsts[i][0:C, 0:C], in_=w_d[:, :])
            nc.scalar.dma_start(out=wsts[i][C:P, P:P + C], in_=w_d[:, :])

        # ---- hidden + query loads, split across SP/ACT dispatch queues
        h_tiles = [hp.tile([P, 8 * P], F32, tag=f"h{g}", name=f"h{g}")
                   for g in range(8)]
        h16_tiles = [hp.tile([P, 8 * P], BF16, tag=f"h16_{g}", name=f"h16_{g}")
                     for g in range(8)]
        for g in range(8):
            eng = nc.sync if g % 2 == 0 else nc.scalar
            eng.dma_start(out=h_tiles[g][:],
                          in_=hidden_r[:, 1024 * g:1024 * (g + 1)])
        q_nat = hp.tile([P, NQC * C], F32, tag="qnat", name="q_nat")
        q16 = hp.tile([P, NQC * C], BF16, tag="q16", name="q16")
        nc.sync.dma_start(out=q_nat[:], in_=query_r[:, :])

        # ---- weight copies (DVE, early: wk2 gates the kT projections)
        for i, wpair in enumerate((wq2, wk2, wv2)):
            wcols = wpair[0].shape[1]
            nc.vector.tensor_copy(out=wpair[0][:], in_=wsts[i][:, 0:wcols])
            nc.vector.tensor_copy(out=wpair[1][:],
                                  in_=wsts[i][:, P:P + wcols])

        # ---- convert hidden to bf16 (DVE-heavy: the ACT copy path is 2x
        # slower and the conversion tail gates the transposes)
        hT_tiles = [htp.tile([P, 4 * P], BF16, tag=f"hT{s}", name=f"hT{s}")
                    for s in range(16)]
        for g in range(8):
            cp = nc.scalar if g in (5, 7) else nc.vector
            _copy(cp, h16_tiles[g][:], h_tiles[g][:])

        # ---- PE warmup: dummy matmuls gated on the first h16 tile, so the
        # ~3.4us HAM un-throttle window completes right as the transposes
        # start (warming earlier is wasted - the gate re-throttles after
        # ~3.4us of idle)
        for i in range(WARMN):
            wt = wpp.tile([P, 512], F32, tag="warm", name="warm")
            nc.tensor.matmul(wt[:], warmw[:],
                             h16_tiles[0][:, 0:512], start=True, stop=True)
        for i4 in range(16):  # quads of 128-col transposes -> one copy each
            pt = tpp.tile([P, 512], BF16, tag="tp", name="pt")
            for k in range(4):
                i = 4 * i4 + k
                g, b = i // 8, i % 8
                nc.tensor.transpose(pt[:, P * k:P * (k + 1)],
                                    h16_tiles[g][:, P * b:P * (b + 1)],
                                    identb[:])
            cp = nc.vector if i4 % 2 == 0 else nc.scalar
            _copy(cp, hT_tiles[i4][:], pt[:])

        # ---- q: convert + transpose (packed like hT) ----
        qT_p = htp.tile([P, 8 * P], BF16, tag="qTp", name="qT_p")
        _copy(nc.vector, q16[:], q_nat[:])
        for i2 in range(4):
            pt = tpp.tile([P, 256], BF16, tag="tp", name="pt")
            for k in range(2):
                i = 2 * i2 + k
                nc.tensor.transpose(pt[:, P * k:P * (k + 1)],
                                    q16[:, P * i:P * (i + 1)], identb[:])
            cp = nc.vector if i2 % 2 == 0 else nc.scalar
            _copy(cp, qT_p[:, 256 * i2:256 * (i2 + 1)], pt[:])

        # ---- kT projection: even chunks via wk2_top, odd via wk2_bot;
        # parity-blocked layout keeps the copies contiguous (tile s =
        # [even chunks 8s+2j | odd chunks 8s+2j+1], 128 cols per chunk)
        for s in range(16):
            for tb in range(2):
                pp = prp.tile([P, 512], F32, tag="pe", name="pp")
                nc.tensor.matmul(pp[:], wk2[tb][:], hT_tiles[s][:],
                                 start=True, stop=True)
                cp = nc.vector if tb == 0 else nc.scalar
                _copy(cp, kT_tiles[s][:, 512 * tb:512 * (tb + 1)], pp[:])

        # ---- qT projection ----
        qdst = qT[:].rearrange("p (i two b) -> p i two b", two=2, b=P)
        for tb in range(2):
            for x in range(2):
                pp = prp.tile([P, 512], F32, tag="pe", name="pp")
                nc.tensor.matmul(pp[:], wq2[tb][:],
                                 qT_p[:, 512 * x:512 * (x + 1)],
                                 start=True, stop=True)
                ppv = pp[:].rearrange("p (i b) -> p i b", b=P)
                cp = nc.vector if (tb + x) % 2 == 0 else nc.scalar
                _copy(cp, qdst[:, 4 * x:4 * x + 4, tb, :], ppv)

        # ---- v: natural layout chunks + ones column ----
        for g in range(8):
            ones_dst = v_tiles[g][:].rearrange(
                "p (k s) -> p k s", s=65)[:, :, C:C + 1]
            nc.vector.tensor_copy(out=ones_dst, in_=ones16[:, :, None])
        # one psum tile per parity: matmuls from different row tile
        # positions must never write the same PSUM tile (hw gotcha)
        for g in range(8):
            for tb in range(2):
                vp = prp.tile([P, 512], F32, tag="po", name="vp")
                for j in range(8):
                    c_ = 16 * g + 2 * j + tb
                    s, jj = c_ // 8, (c_ % 8) // 2
                    nc.tensor.matmul(
                        vp[:, C * j:C * (j + 1)],
                        hT_tiles[s][:, P * jj:P * (jj + 1)],
                        wv2[tb][:],
                        start=True, stop=True)
                dv = v_tiles[g][:].rearrange("p (k two s) -> p k two s",
                                             two=2, s=65)
                vpv = vp[:].rearrange("p (k c) -> p k c", c=C)
                cp = nc.vector if tb == 0 else nc.scalar
                _copy(cp, dv[:, :, tb, 0:C], vpv)

    # ---------------- steady state ----------------
    scp = ctx_.enter_context(tc.tile_pool(name="sc_psum", bufs=SCB, space="PSUM"))
    scpb = ctx_.enter_context(tc.tile_pool(name="scb_psum", bufs=SCB, space="PSUM"))
    otp = ctx_.enter_context(tc.tile_pool(name="ot_psum", bufs=1, space="PSUM"))
    ptp = ctx_.enter_context(tc.tile_pool(name="pt_psum", bufs=2, space="PSUM"))
    # separate pools for the ACT- and DVE-written exp halves: a shared tile
    # would serialize the two engines via co-writer/co-reader ordering
    xpa = ctx_.enter_context(tc.tile_pool(name="expA", bufs=XB))
    xpb = ctx_.enter_context(tc.tile_pool(name="expB", bufs=XB))
    eps = ctx_.enter_context(tc.tile_pool(name="epi", bufs=4))

    for h in range(NPASS_RUN):
        q0 = QPASS * h
        outT = otp.tile([P, QPASS], F32, tag="outT")

        def s1(c, scab):
            # parity-blocked kT layout: tile c//8, even chunks in cols
            # [0:512), odd in [512:1024), 128 cols per chunk
            k0 = 512 * (c % 2) + P * ((c % 8) // 2)
            kt = kT_tiles[c // 8][:, k0:k0 + P]
            for x in range(2):
                nc.tensor.matmul(
                    scab[x][:],
                    kt[:, :],
                    qT[:, q0 + 512 * x:q0 + 512 * (x + 1)],
                    start=True, stop=True)

        def sc_pair():
            a = scp.tile([P, 512], F32, tag="scA", name="scA")
            b = scpb.tile([P, 512], F32, tag="scB", name="scB")
            return (a, b)

        sc_cur = sc_pair()
        s1(0, sc_cur)
        for c in range(NKVC_RUN):
            # exp split across two engines so neither stalls the PE (which
            # must stream gap-free to stay at the warm 2.4 GHz clock):
            # ACT does the exact exp on cols [0:512), DVE emits the
            # Schraudolph bf16 bit-pattern on cols [512:1024).
            expA = xpa.tile([P, 512], BF16, tag="expA")
            expB = xpb.tile([P, 512], BF16, tag="expB")
            nc.scalar.activation(expA[:], sc_cur[0][:], AF.Exp, scale=0.125)
            nc.vector.tensor_scalar(
                out=expB[:].bitcast(I16),
                in0=sc_cur[1][:],
                scalar1=SCH_A, scalar2=SCH_B,
                op0=mybir.AluOpType.mult, op1=mybir.AluOpType.add)
            if c + 1 < NKVC_RUN:
                sc_nxt = sc_pair()
                s1(c + 1, sc_nxt)
                sc_cur = sc_nxt
            vt = v_tiles[c // 16][:, 65 * (c % 16):65 * (c % 16) + 65]
            for x, ex in enumerate((expA, expB)):
                nc.tensor.matmul(
                    outT[0:65, 512 * x:512 * (x + 1)],
                    vt,
                    ex[:],
                    start=(c == 0), stop=(c == NKVC_RUN - 1))

        # ---- epilogue for this pass: transpose blocks (packed 4 per PSUM
        # tile so they stream back-to-back instead of ring-stalling on the
        # muls), normalize by the ones-row sum (alternating DVE/ACT), one
        # contiguous out-DMA
        oT_sb = eps.tile([P, QPASS], F32, tag="oT_sb")
        nc.vector.tensor_copy(out=oT_sb[0:65, 0:512], in_=outT[0:65, 0:512])
        nc.scalar.copy(out=oT_sb[0:65, 512:1024], in_=outT[0:65, 512:1024])
        resbig = eps.tile([P, 512], F32, tag="resbig")
        pts = []
        for q4 in range(2):
            pt4 = ptp.tile([P, 4 * 65], F32, tag="pt")
            pts.append(pt4)
            for k in range(4):
                j2 = 4 * q4 + k
                nc.tensor.transpose(pt4[:, 65 * k:65 * k + 65],
                                    oT_sb[0:65, P * j2:P * (j2 + 1)],
                                    identf[0:65, 0:65])
        for j2 in range(8):
            pt = pts[j2 // 4][:, 65 * (j2 % 4):65 * (j2 % 4) + 65]
            rcp = eps.tile([P, 1], F32, tag="rcp")
            nc.vector.reciprocal(rcp[:], pt[:, C:C + 1])
            dst = resbig[:, C * j2:C * (j2 + 1)]
            if j2 % 2 == 0:
                nc.vector.tensor_scalar_mul(dst, pt[:, 0:C], rcp[:])
            else:
                nc.scalar.mul(dst, pt[:, 0:C], rcp[:])
        eng = nc.sync if h % 2 == 0 else nc.scalar
        eng.dma_start(out=out_flat[:, 512 * h:512 * (h + 1)], in_=resbig[:])


def build_program(repeat=1):
    from contextlib import ExitStack

    nc = bacc.Bacc("TRN2", target_bir_lowering=False, debug=False,
                   num_devices=NCORES)
    hidden_d = nc.dram_tensor("hidden", [NKV, C], F32, kind="ExternalInput").ap()
    query_d = nc.dram_tensor("query", [NQ, C], F32, kind="ExternalInput").ap()
    wq_d = nc.dram_tensor("Wq", [C, C], F32, kind="ExternalInput").ap()
    wk_d = nc.dram_tensor("Wk", [C, C], F32, kind="ExternalInput").ap()
    wv_d = nc.dram_tensor("Wv", [C, C], F32, kind="ExternalInput").ap()
    out_d = nc.dram_tensor("out", [NQ, C], F32, kind="ExternalOutput").ap()

    with tile.TileContext(nc) as tc:
        with ExitStack() as ctx_:
            if repeat == 1:
                _emit(nc, tc, ctx_, hidden_d, query_d, wq_d, wk_d, wv_d,
                      out_d)
            else:
                with tc.For_i(0, repeat, 1):
                    _emit(nc, tc, ctx_, hidden_d, query_d, wq_d, wk_d, wv_d,
                          out_d)
    nc.compile()
    return nc


_NC_CACHE = {}


def kernel(hidden, query, Wq, Wk, Wv):
    hidden = np.ascontiguousarray(np.asarray(hidden, dtype=np.float32))
    query = np.ascontiguousarray(np.asarray(query, dtype=np.float32))
    Wq = np.ascontiguousarray(np.asarray(Wq, dtype=np.float32))
    Wk = np.ascontiguousarray(np.asarray(Wk, dtype=np.float32))
    Wv = np.ascontiguousarray(np.asarray(Wv, dtype=np.float32))

    if "nc" not in _NC_CACHE:
        _NC_CACHE["nc"] = build_program()
    nc = _NC_CACHE["nc"]

    in_maps = [
        {"hidden": hidden, "query": query[i * NQ:(i + 1) * NQ],
         "Wq": Wq, "Wk": Wk, "Wv": Wv}
        for i in range(NCORES)
    ]
    res = run_bass_kernel_spmd(nc, in_maps, core_ids=list(range(NCORES)))
    return np.concatenate([res.results[i]["out"] for i in range(NCORES)],
                          axis=0)


# revision 32
# speedup vs baseline: 1.0073x; 1.0073x over previous
"""Trainium2 Bass kernel for sparse-attention (full 16384x16384 attention,
64-dim head, 64x64 projections).

Sharding: query rows split across 8 cores (2048 rows each); hidden (K/V
source) and the 64x64 projection weights are replicated on every core.

Per-core algorithm:
  prologue (all-bf16, all matmuls use full-128-row stationaries so the PE
  HAM clock gate sees a busy array and runs at 2.4 GHz):
    warmup   ~16 dummy matmuls gated on the first h16 tile: un-throttle the
             PE HAM clock gate right before the real matmuls start
    h16      hidden loaded f32 (DMAs split across the SP and ACT dispatch
             queues), converted to bf16 (DVE-heavy)
    hT       PE-transposed 128x128 blocks of h16; 16 packed tiles [128,512],
             top half = even kv chunks' hidden^T, bottom = odd chunks'
    kT       k^T via zero-padded stationaries wk2_top=[[Wk],[0]],
             wk2_bot=[[0],[Wk]] (K=128 keeps the HAM warm; the zero half
             multiplies the other parity's data away exactly).  Stored
             parity-blocked (tile s = [even chunks | odd chunks]) so the
             PSUM->SBUF copies stay contiguous.
    qT       q^T likewise [128, 2048] (rows 64:128 exact zeros)
    v        v in natural [kv, c] chunks, each padded with a ones column
             (the ones column makes stage 2 accumulate the softmax
             denominator as row 64 of outT)
  steady state, per q-pass (1024 q cols) x 128 kv chunks, software-
  pipelined so the PE streams gap-free at 2.4 GHz (863 ns/chunk):
    stage1: scA[kv=128,512], scB[kv=128,512] = kT_chunk^T @ qT  (PE->PSUM)
    exp:    expA = exp(scA/8) on ACT; expB = Schraudolph bf16 bit-pattern
            round(A*scB+B) as int16 on DVE.  Two engines in parallel, each
            under the PE's 864 ns/chunk budget; fully separate tiles so the
            tile framework cannot serialize them.
    stage2: outT[65, q] += v_chunk_aug^T @ exp{A,B}            (PE, PSUM)
  epilogue per pass: outT -> SBUF (copy split ACT/DVE), PE-transpose
  128-col blocks, multiply by reciprocal of the ones-column sum
  (alternating DVE/ACT), DMA out.

kv ordering is permuted (chunk c holds kv rows {128p + c}) - harmless since
softmax fully reduces over kv and k/v use the same permutation.  q ordering:
column g = 128t + p corresponds to query row 16p + t of this core's shard.
"""

import numpy as np

import concourse.bass as bass
import concourse.mybir as mybir
import concourse.tile as tile
from concourse import bacc
from concourse.bass_utils import run_bass_kernel_spmd
from concourse.masks import make_identity

F32 = mybir.dt.float32
BF16 = mybir.dt.bfloat16
I16 = mybir.dt.int16
AF = mybir.ActivationFunctionType

# Schraudolph exp for bf16: bitpattern(exp(s/8)) ~= round(A*s + B).
# A = 2^7 * log2(e) / 8 (folds the 1/sqrt(64) score scale); B centers the
# exponent at 127 with c=0.04368 minimizing max relative error (~3%).
SCH_A = 128.0 * 0.125 * 1.4426950408889634
SCH_B = 128.0 * (127.0 - 0.04368)

NKV = 16384
C = 64
NCORES = 8
NQ = NKV // NCORES  # 2048 q rows per core
P = 128

NKVC = NKV // P      # 128 kv chunks
NQC = NQ // P        # 16 q chunks
QPASS = 1024         # q columns processed per pass
NPASS = NQ // QPASS  # 2

import os
NKVC_RUN = int(os.environ.get("NKVC_RUN", NKVC))
NPASS_RUN = int(os.environ.get("NPASS_RUN", NPASS))
SCB = int(os.environ.get("SCB", "2"))
XB = int(os.environ.get("XB", "4"))
WARMN = int(os.environ.get("WARMN", "16"))


def _copy(eng, out, in_):
    if hasattr(eng, "tensor_copy"):
        eng.tensor_copy(out=out, in_=in_)
    else:
        eng.copy(out=out, in_=in_)


def _emit(nc, tc, ctx_, hidden_d, query_d, wq_d, wk_d, wv_d, out_d):
    from contextlib import ExitStack

    # ---------------- constants ----------------
    consts = ctx_.enter_context(tc.tile_pool(name="consts", bufs=1))
    # top/bot variants: w*t = [[W],[0]], w*b = [[0],[W]] so projections from
    # the parity-packed hT tiles contract over all 128 partitions (the zero
    # half exactly cancels the other parity's rows)
    wq2 = [consts.tile([P, P], BF16, name=f"wq2{i}") for i in range(2)]
    wk2 = [consts.tile([P, P], BF16, name=f"wk2{i}") for i in range(2)]
    wv2 = [consts.tile([P, C], BF16, name=f"wv2{i}") for i in range(2)]
    # one staging tile per weight: a shared one serializes DMA->copy->DMA
    wsts = [consts.tile([P, 2 * P], F32, name=f"wst{i}") for i in range(3)]
    ones16 = consts.tile([P, 16], BF16, name="ones16")
    identb = consts.tile([P, P], BF16, name="identb")
    identf = consts.tile([P, P], F32, name="identf")
    warmw = consts.tile([P, P], BF16, name="warmw")
    warmm = consts.tile([P, 512], BF16, name="warmm")

    nc.vector.memset(warmw[:], 0.0)
    nc.vector.memset(warmm[:], 0.0)
    nc.vector.memset(ones16[:], 1.0)
    make_identity(nc, identb)
    make_identity(nc, identf)

    # persistent operand tiles
    big = ctx_.enter_context(tc.tile_pool(name="big", bufs=1))
    kT_tiles = [big.tile([P, 8 * P], BF16, tag=f"kt{i}", name=f"kt{i}")
                for i in range(16)]
    qT = big.tile([P, NQ], BF16, tag="qt", name="qT")
    v_tiles = [big.tile([P, 16 * 65], BF16, tag=f"v{i}", name=f"v{i}")
               for i in range(8)]

    hidden_r = hidden_d.rearrange("(p t) c -> p (t c)", p=P)   # [128, 8192]
    query_r = query_d.rearrange("(p t) c -> p (t c)", p=P)     # [128, 1024]
    out_flat = out_d.rearrange("(p t) c -> p (t c)", p=P)      # [128, 1024]

    with ExitStack() as pro:
        hp = pro.enter_context(tc.tile_pool(name="hnat", bufs=1))
        htp = pro.enter_context(tc.tile_pool(name="ht", bufs=1))
        tpp = pro.enter_context(
            tc.tile_pool(name="tp_psum", bufs=2, space="PSUM"))
        prp = pro.enter_context(
            tc.tile_pool(name="proj_psum", bufs=2, space="PSUM"))
        wpp = pro.enter_context(
            tc.tile_pool(name="warm_psum", bufs=2, space="PSUM"))

        # ---- weight DMAs first: tiny (16KB), and wk2 gates the projection
        # chain; separate stagings avoid DMA->copy->DMA serialization
        for i, w_d in enumerate((wq_d, wk_d, wv_d)):
            nc.vector.memset(wsts[i][:], 0.0)
            nc.sync.dma_start(out=wsts[i][0:C, 0:C], in_=w_d[:, :])
            nc.scalar.dma_start(out=wsts[i][C:P, P:P + C], in_=w_d[:, :])

        # ---- hidden + query loads, split across SP/ACT dispatch queues
        h_tiles = [hp.tile([P, 8 * P], F32, tag=f"h{g}", name=f"h{g}")
                   for g in range(8)]
        h16_tiles = [hp.tile([P, 8 * P], BF16, tag=f"h16_{g}", name=f"h16_{g}")
                     for g in range(8)]
        for g in range(8):
            eng = nc.sync if g % 2 == 0 else nc.scalar
            eng.dma_start(out=h_tiles[g][:],
                          in_=hidden_r[:, 1024 * g:1024 * (g + 1)])
        q_nat = hp.tile([P, NQC * C], F32, tag="qnat", name="q_nat")
        q16 = hp.tile([P, NQC * C], BF16, tag="q16", name="q16")
        nc.sync.dma_start(out=q_nat[:], in_=query_r[:, :])

        # ---- weight copies (DVE, early: wk2 gates the kT projections)
        for i, wpair in enumerate((wq2, wk2, wv2)):
            wcols = wpair[0].shape[1]
            nc.vector.tensor_copy(out=wpair[0][:], in_=wsts[i][:, 0:wcols])
            nc.vector.tensor_copy(out=wpair[1][:],
                                  in_=wsts[i][:, P:P + wcols])

        # ---- convert hidden to bf16 (DVE-heavy: the ACT copy path is 2x
        # slower and the conversion tail gates the transposes)
        hT_tiles = [htp.tile([P, 4 * P], BF16, tag=f"hT{s}", name=f"hT{s}")
                    for s in range(16)]
        for g in range(8):
            cp = nc.scalar if g in (5, 7) else nc.vector
            _copy(cp, h16_tiles[g][:], h_tiles[g][:])

        # ---- PE warmup: dummy matmuls gated on the first h16 tile, so the
        # ~3.4us HAM un-throttle window completes right as the transposes
        # start (warming earlier is wasted - the gate re-throttles after
        # ~3.4us of idle)
        for i in range(WARMN):
            wt = wpp.tile([P, 512], F32, tag="warm", name="warm")
            nc.tensor.matmul(wt[:], warmw[:],
                             h16_tiles[0][:, 0:512], start=True, stop=True)
        for i4 in range(16):  # quads of 128-col transposes -> one copy each
            pt = tpp.tile([P, 512], BF16, tag="tp", name="pt")
            for k in range(4):
                i = 4 * i4 + k
                g, b = i // 8, i % 8
                nc.tensor.transpose(pt[:, P * k:P * (k + 1)],
                                    h16_tiles[g][:, P * b:P * (b + 1)],
                                    identb[:])
            cp = nc.vector if i4 % 2 == 0 else nc.scalar
            _copy(cp, hT_tiles[i4][:], pt[:])

        # ---- q: convert + transpose (packed like hT) ----
        qT_p = htp.tile([P, 8 * P], BF16, tag="qTp", name="qT_p")
        _copy(nc.vector, q16[:], q_nat[:])
        for i2 in range(4):
            pt = tpp.tile([P, 256], BF16, tag="tp", name="pt")
            for k in range(2):
                i = 2 * i2 + k
                nc.tensor.transpose(pt[:, P * k:P * (k + 1)],
                                    q16[:, P * i:P * (i + 1)], identb[:])
            cp = nc.vector if i2 % 2 == 0 else nc.scalar
            _copy(cp, qT_p[:, 256 * i2:256 * (i2 + 1)], pt[:])

        # ---- kT projection: even chunks via wk2_top, odd via wk2_bot;
        # parity-blocked layout keeps the copies contiguous (tile s =
        # [even chunks 8s+2j | odd chunks 8s+2j+1], 128 cols per chunk)
        for s in range(16):
            for tb in range(2):
                pp = prp.tile([P, 512], F32, tag="pe", name="pp")
                nc.tensor.matmul(pp[:], wk2[tb][:], hT_tiles[s][:],
                                 start=True, stop=True)
                cp = nc.vector if tb == 0 else nc.scalar
                _copy(cp, kT_tiles[s][:, 512 * tb:512 * (tb + 1)], pp[:])

        # ---- qT projection ----
        qdst = qT[:].rearrange("p (i two b) -> p i two b", two=2, b=P)
        for tb in range(2):
            for x in range(2):
                pp = prp.tile([P, 512], F32, tag="pe", name="pp")
                nc.tensor.matmul(pp[:], wq2[tb][:],
                                 qT_p[:, 512 * x:512 * (x + 1)],
                                 start=True, stop=True)
                ppv = pp[:].rearrange("p (i b) -> p i b", b=P)
                cp = nc.vector if (tb + x) % 2 == 0 else nc.scalar
                _copy(cp, qdst[:, 4 * x:4 * x + 4, tb, :], ppv)

        # ---- v: natural layout chunks + ones column ----
        for g in range(8):
            ones_dst = v_tiles[g][:].rearrange(
                "p (k s) -> p k s", s=65)[:, :, C:C + 1]
            nc.vector.tensor_copy(out=ones_dst, in_=ones16[:, :, None])
        # one psum tile per parity: matmuls from different row tile
        # positions must never write the same PSUM tile (hw gotcha)
        for g in range(8):
            for tb in range(2):
                vp = prp.tile([P, 512], F32, tag="po", name="vp")
                for j in range(8):
                    c_ = 16 * g + 2 * j + tb
                    s, jj = c_ // 8, (c_ % 8) // 2
                    nc.tensor.matmul(
                        vp[:, C * j:C * (j + 1)],
                        hT_tiles[s][:, P * jj:P * (jj + 1)],
                        wv2[tb][:],
                        start=True, stop=True)
                dv = v_tiles[g][:].rearrange("p (k two s) -> p k two s",
                                             two=2, s=65)
                vpv = vp[:].rearrange("p (k c) -> p k c", c=C)
                cp = nc.vector if tb == 0 else nc.scalar
                _copy(cp, dv[:, :, tb, 0:C], vpv)

    # ---------------- steady state ----------------
    scp = ctx_.enter_context(tc.tile_pool(name="sc_psum", bufs=SCB, space="PSUM"))
    scpb = ctx_.enter_context(tc.tile_pool(name="scb_psum", bufs=SCB, space="PSUM"))
    otp = ctx_.enter_context(tc.tile_pool(name="ot_psum", bufs=1, space="PSUM"))
    ptp = ctx_.enter_context(tc.tile_pool(name="pt_psum", bufs=2, space="PSUM"))
    # separate pools for the ACT- and DVE-written exp halves: a shared tile
    # would serialize the two engines via co-writer/co-reader ordering
    xpa = ctx_.enter_context(tc.tile_pool(name="expA", bufs=XB))
    xpb = ctx_.enter_context(tc.tile_pool(name="expB", bufs=XB))
    eps = ctx_.enter_context(tc.tile_pool(name="epi", bufs=4))

    for h in range(NPASS_RUN):
        q0 = QPASS * h
        outT = otp.tile([P, QPASS], F32, tag="outT")

        def s1(c, scab):
            # parity-blocked kT layout: tile c//8, even chunks in cols
            # [0:512), odd in [512:1024), 128 cols per chunk
            k0 = 512 * (c % 2) + P * ((c % 8) // 2)
            kt = kT_tiles[c // 8][:, k0:k0 + P]
            for x in range(2):
                nc.tensor.matmul(
                    scab[x][:],
                    kt[:, :],
                    qT[:, q0 + 512 * x:q0 + 512 * (x + 1)],
                    start=True, stop=True)

        def sc_pair():
            a = scp.tile([P, 512], F32, tag="scA", name="scA")
            b = scpb.tile([P, 512], F32, tag="scB", name="scB")
            return (a, b)

        sc_cur = sc_pair()
        s1(0, sc_cur)
        for c in range(NKVC_RUN):
            # exp split across two engines so neither stalls the PE (which
            # must stream gap-free to stay at the warm 2.4 GHz clock):
            # ACT does the exact exp on cols [0:512), DVE emits the
            # Schraudolph bf16 bit-pattern on cols [512:1024).
            expA = xpa.tile([P, 512], BF16, tag="expA")
            expB = xpb.tile([P, 512], BF16, tag="expB")
            nc.scalar.activation(expA[:], sc_cur[0][:], AF.Exp, scale=0.125)
            nc.vector.tensor_scalar(
                out=expB[:].bitcast(I16),
                in0=sc_cur[1][:],
                scalar1=SCH_A, scalar2=SCH_B,
                op0=mybir.AluOpType.mult, op1=mybir.AluOpType.add)
            if c + 1 < NKVC_RUN:
                sc_nxt = sc_pair()
                s1(c + 1, sc_nxt)
                sc_cur = sc_nxt
            vt = v_tiles[c // 16][:, 65 * (c % 16):65 * (c % 16) + 65]
            for x, ex in enumerate((expA, expB)):
                nc.tensor.matmul(
                    outT[0:65, 512 * x:512 * (x + 1)],
                    vt,
                    ex[:],
                    start=(c == 0), stop=(c == NKVC_RUN - 1))

        # ---- epilogue for this pass: transpose blocks (packed 4 per PSUM
        # tile so they stream back-to-back instead of ring-stalling on the
        # muls), normalize by the ones-row sum (alternating DVE/ACT), one
        # contiguous out-DMA
        oT_sb = eps.tile([P, QPASS], F32, tag="oT_sb")
        nc.vector.tensor_copy(out=oT_sb[0:65, 0:512], in_=outT[0:65, 0:512])
        nc.scalar.copy(out=oT_sb[0:65, 512:1024], in_=outT[0:65, 512:1024])
        resbig = eps.tile([P, 512], F32, tag="resbig")
        pts = []
        for q4 in range(2):
            pt4 = ptp.tile([P, 4 * 65], F32, tag="pt")
            pts.append(pt4)
            for k in range(4):
                j2 = 4 * q4 + k
                nc.tensor.transpose(pt4[:, 65 * k:65 * k + 65],
                                    oT_sb[0:65, P * j2:P * (j2 + 1)],
                                    identf[0:65, 0:65])
        for j2 in range(8):
            pt = pts[j2 // 4][:, 65 * (j2 % 4):65 * (j2 % 4) + 65]
            rcp = eps.tile([P, 1], F32, tag="rcp")
            nc.vector.reciprocal(rcp[:], pt[:, C:C + 1])
            dst = resbig[:, C * j2:C * (j2 + 1)]
            if j2 % 2 == 0:
                nc.vector.tensor_scalar_mul(dst, pt[:, 0:C], rcp[:])
            else:
                nc.scalar.mul(dst, pt[:, 0:C], rcp[:])
        eng = nc.sync if h % 2 == 0 else nc.scalar
        eng.dma_start(out=out_flat[:, 512 * h:512 * (h + 1)], in_=resbig[:])


def build_program(repeat=1):
    from contextlib import ExitStack

    nc = bacc.Bacc("TRN2", target_bir_lowering=False, debug=False,
                   num_devices=NCORES)
    hidden_d = nc.dram_tensor("hidden", [NKV, C], F32, kind="ExternalInput").ap()
    query_d = nc.dram_tensor("query", [NQ, C], F32, kind="ExternalInput").ap()
    wq_d = nc.dram_tensor("Wq", [C, C], F32, kind="ExternalInput").ap()
    wk_d = nc.dram_tensor("Wk", [C, C], F32, kind="ExternalInput").ap()
    wv_d = nc.dram_tensor("Wv", [C, C], F32, kind="ExternalInput").ap()
    out_d = nc.dram_tensor("out", [NQ, C], F32, kind="ExternalOutput").ap()

    with tile.TileContext(nc) as tc:
        with ExitStack() as ctx_:
            if repeat == 1:
                _emit(nc, tc, ctx_, hidden_d, query_d, wq_d, wk_d, wv_d,
                      out_d)
            else:
                with tc.For_i(0, repeat, 1):
                    _emit(nc, tc, ctx_, hidden_d, query_d, wq_d, wk_d, wv_d,
                          out_d)
    nc.compile()
    return nc


_NC_CACHE = {}


def kernel(hidden, query, Wq, Wk, Wv):
    hidden = np.ascontiguousarray(np.asarray(hidden, dtype=np.float32))
    query = np.ascontiguousarray(np.asarray(query, dtype=np.float32))
    Wq = np.ascontiguousarray(np.asarray(Wq, dtype=np.float32))
    Wk = np.ascontiguousarray(np.asarray(Wk, dtype=np.float32))
    Wv = np.ascontiguousarray(np.asarray(Wv, dtype=np.float32))

    if "nc" not in _NC_CACHE:
        _NC_CACHE["nc"] = build_program()
    nc = _NC_CACHE["nc"]

    in_maps = [
        {"hidden": hidden, "query": query[i * NQ:(i + 1) * NQ],
         "Wq": Wq, "Wk": Wk, "Wv": Wv}
        for i in range(NCORES)
    ]
    res = run_bass_kernel_spmd(nc, in_maps, core_ids=list(range(NCORES)))
    return np.concatenate([res.results[i]["out"] for i in range(NCORES)],
                          axis=0)


# revision 39
# speedup vs baseline: 1.0158x; 1.0085x over previous
"""Trainium2 Bass kernel for sparse-attention (full 16384x16384 attention,
64-dim head, 64x64 projections).

Sharding: query rows split across 8 cores (2048 rows each); hidden (K/V
source) and the 64x64 projection weights are replicated on every core.

Per-core algorithm:
  prologue (all-bf16, all matmuls use full-128-row stationaries so the PE
  HAM clock gate sees a busy array and runs at 2.4 GHz):
    warmup   ~16 dummy matmuls gated on the first h16 tile: un-throttle the
             PE HAM clock gate right before the real matmuls start
    h16      hidden loaded f32 (DMAs split across the SP and ACT dispatch
             queues), converted to bf16 (DVE-heavy)
    hT       PE-transposed 128x128 blocks of h16; 16 packed tiles [128,512],
             top half = even kv chunks' hidden^T, bottom = odd chunks'.
             These double as the stage-1 stationaries: no k is ever
             materialized, because S^T = H Wk Wq^T Q^T = H @ (Wk q'^T).
    qT,rq    q'^T = Wq^T Q^T [128, 2048] (rows 64:128 exact zeros), then
             rq[parity] = Wk q'^T with the 64 data rows placed at
             partitions 0:64 (even kv chunks) or 64:128 (odd) and exact
             zeros elsewhere, so contraction with the parity-packed hT
             stationary picks out exactly one chunk (K=128 keeps the HAM
             clock gate warm).
    v        v in natural [kv, c] chunks, each padded with a ones column
             (the ones column makes stage 2 accumulate the softmax
             denominator as row 64 of outT)
  steady state, per q-pass (1024 q cols) x 128 kv chunks, software-
  pipelined so the PE streams gap-free at 2.4 GHz (863 ns/chunk):
    stage1: scA[kv=128,512], scB[kv=128,512] = hT_blk^T @ rq  (PE->PSUM)
    exp:    expA = exp(scA/8) on ACT; expB = Schraudolph bf16 bit-pattern
            round(A*scB+B) as int16 on DVE.  Two engines in parallel, each
            under the PE's 864 ns/chunk budget; fully separate tiles so the
            tile framework cannot serialize them.
    stage2: outT[65, q] += v_chunk_aug^T @ exp{A,B}            (PE, PSUM)
  epilogue per pass: outT -> SBUF (copy split ACT/DVE), PE-transpose
  128-col blocks, multiply by reciprocal of the ones-column sum
  (alternating DVE/ACT), DMA out.

kv ordering is permuted (chunk c holds kv rows {128p + c}) - harmless since
softmax fully reduces over kv and k/v use the same permutation.  q ordering:
column g = 128t + p corresponds to query row 16p + t of this core's shard.
"""

import numpy as np

import concourse.bass as bass
import concourse.mybir as mybir
import concourse.tile as tile
from concourse import bacc
from concourse.bass_utils import run_bass_kernel_spmd
from concourse.masks import make_identity

F32 = mybir.dt.float32
BF16 = mybir.dt.bfloat16
I16 = mybir.dt.int16
AF = mybir.ActivationFunctionType

# Schraudolph exp for bf16: bitpattern(exp(s/8)) ~= round(A*s + B).
# A = 2^7 * log2(e) / 8 (folds the 1/sqrt(64) score scale); B centers the
# exponent at 127 with c=0.04368 minimizing max relative error (~3%).
SCH_A = 128.0 * 0.125 * 1.4426950408889634
SCH_B = 128.0 * (127.0 - 0.04368)

NKV = 16384
C = 64
NCORES = 8
NQ = NKV // NCORES  # 2048 q rows per core
P = 128

NKVC = NKV // P      # 128 kv chunks
NQC = NQ // P        # 16 q chunks
QPASS = 1024         # q columns processed per pass
NPASS = NQ // QPASS  # 2

import os
NKVC_RUN = int(os.environ.get("NKVC_RUN", NKVC))
NPASS_RUN = int(os.environ.get("NPASS_RUN", NPASS))
SCB = int(os.environ.get("SCB", "2"))
XB = int(os.environ.get("XB", "4"))
WARMN = int(os.environ.get("WARMN", "16"))


def _copy(eng, out, in_):
    if hasattr(eng, "tensor_copy"):
        eng.tensor_copy(out=out, in_=in_)
    else:
        eng.copy(out=out, in_=in_)


def _emit(nc, tc, ctx_, hidden_d, query_d, wq_d, wk_d, wv_d, out_d):
    from contextlib import ExitStack

    # ---------------- constants ----------------
    consts = ctx_.enter_context(tc.tile_pool(name="consts", bufs=1))
    # top/bot variants: w*t = [[W],[0]], w*b = [[0],[W]] so projections from
    # the parity-packed hT tiles contract over all 128 partitions (the zero
    # half exactly cancels the other parity's rows)
    wq2 = [consts.tile([P, P], BF16, name=f"wq2{i}") for i in range(2)]
    # Wk is folded into the q side: S^T = H (Wk q'^T), so stage 1 uses the
    # hT tiles directly as stationary and no kT is ever materialized.
    # wkt[0] places Wk^T at out-partitions 0:64 (for even kv chunks),
    # wkt[1] at 64:128 (odd chunks); all other entries zero.
    wkt = [consts.tile([P, P], BF16, name=f"wkt{i}") for i in range(2)]
    wv2 = [consts.tile([P, C], BF16, name=f"wv2{i}") for i in range(2)]
    # one staging tile per weight: a shared one serializes DMA->copy->DMA
    wsts = [consts.tile([P, 2 * P], F32, name=f"wst{i}") for i in range(3)]
    ones16 = consts.tile([P, 16], BF16, name="ones16")
    identb = consts.tile([P, P], BF16, name="identb")
    identf = consts.tile([P, P], F32, name="identf")
    warmw = consts.tile([P, P], BF16, name="warmw")
    warmm = consts.tile([P, 512], BF16, name="warmm")

    nc.vector.memset(warmw[:], 0.0)
    nc.vector.memset(warmm[:], 0.0)
    nc.vector.memset(ones16[:], 1.0)
    nc.vector.memset(wkt[0][:], 0.0)
    nc.vector.memset(wkt[1][:], 0.0)
    make_identity(nc, identb)
    make_identity(nc, identf)

    # persistent operand tiles (hT doubles as the stage-1 stationary)
    big = ctx_.enter_context(tc.tile_pool(name="big", bufs=1))
    hT_tiles = [big.tile([P, 4 * P], BF16, tag=f"hT{s}", name=f"hT{s}")
                for s in range(16)]
    qT = big.tile([P, NQ], BF16, tag="qt", name="qT")
    # rq[parity] = (Wk q'^T) with the data on partitions 0:64 (even kv
    # chunks) or 64:128 (odd); the other half is exact zeros so contraction
    # with the parity-packed hT stationary picks out one chunk.
    rq = [big.tile([P, NQ], BF16, tag=f"rq{i}", name=f"rq{i}")
          for i in range(2)]
    v_tiles = [big.tile([P, 16 * 65], BF16, tag=f"v{i}", name=f"v{i}")
               for i in range(8)]

    hidden_r = hidden_d.rearrange("(p t) c -> p (t c)", p=P)   # [128, 8192]
    query_r = query_d.rearrange("(p t) c -> p (t c)", p=P)     # [128, 1024]
    out_flat = out_d.rearrange("(p t) c -> p (t c)", p=P)      # [128, 1024]

    with ExitStack() as pro:
        hp = pro.enter_context(tc.tile_pool(name="hnat", bufs=1))
        htp = pro.enter_context(tc.tile_pool(name="ht", bufs=1))
        tpp = pro.enter_context(
            tc.tile_pool(name="tp_psum", bufs=2, space="PSUM"))
        prp = pro.enter_context(
            tc.tile_pool(name="proj_psum", bufs=2, space="PSUM"))
        wpp = pro.enter_context(
            tc.tile_pool(name="warm_psum", bufs=2, space="PSUM"))

        # ---- weight DMAs first: tiny (16KB), and they gate the projection
        # chain; separate stagings avoid DMA->copy->DMA serialization
        for i, w_d in enumerate((wq_d, wk_d, wv_d)):
            nc.vector.memset(wsts[i][:], 0.0)
            nc.sync.dma_start(out=wsts[i][0:C, 0:C], in_=w_d[:, :])
            if i != 1:  # Wk is only needed once (transposed on the PE)
                nc.scalar.dma_start(out=wsts[i][C:P, P:P + C], in_=w_d[:, :])

        # ---- hidden + query loads, split across SP/ACT dispatch queues
        h_tiles = [hp.tile([P, 8 * P], F32, tag=f"h{g}", name=f"h{g}")
                   for g in range(8)]
        h16_tiles = [hp.tile([P, 8 * P], BF16, tag=f"h16_{g}", name=f"h16_{g}")
                     for g in range(8)]
        for g in range(8):
            eng = nc.sync if g % 2 == 0 else nc.scalar
            eng.dma_start(out=h_tiles[g][:],
                          in_=hidden_r[:, 1024 * g:1024 * (g + 1)])
        q_nat = hp.tile([P, NQC * C], F32, tag="qnat", name="q_nat")
        q16 = hp.tile([P, NQC * C], BF16, tag="q16", name="q16")
        nc.sync.dma_start(out=q_nat[:], in_=query_r[:, :])

        # ---- weight copies (DVE, early: they gate the projections)
        for i, wpair in ((0, wq2), (2, wv2)):
            wcols = wpair[0].shape[1]
            nc.vector.tensor_copy(out=wpair[0][:], in_=wsts[i][:, 0:wcols])
            nc.vector.tensor_copy(out=wpair[1][:],
                                  in_=wsts[i][:, P:P + wcols])

        # ---- convert hidden to bf16 (DVE-heavy: the ACT copy path is 2x
        # slower and the conversion tail gates the transposes)
        for g in range(8):
            cp = nc.scalar if g in (5, 7) else nc.vector
            _copy(cp, h16_tiles[g][:], h_tiles[g][:])

        # ---- PE warmup: dummy matmuls gated on the first h16 tile, so the
        # ~3.4us HAM un-throttle window completes right as the transposes
        # start (warming earlier is wasted - the gate re-throttles after
        # ~3.4us of idle)
        for i in range(WARMN):
            wt = wpp.tile([P, 512], F32, tag="warm", name="warm")
            nc.tensor.matmul(wt[:], warmw[:],
                             h16_tiles[0][:, 0:512], start=True, stop=True)
        for i4 in range(16):  # quads of 128-col transposes -> one copy each
            pt = tpp.tile([P, 512], BF16, tag="tp", name="pt")
            for k in range(4):
                i = 4 * i4 + k
                g, b = i // 8, i % 8
                nc.tensor.transpose(pt[:, P * k:P * (k + 1)],
                                    h16_tiles[g][:, P * b:P * (b + 1)],
                                    identb[:])
            cp = nc.vector if i4 % 2 == 0 else nc.scalar
            _copy(cp, hT_tiles[i4][:], pt[:])

        # ---- q: convert + transpose (packed like hT) ----
        qT_p = htp.tile([P, 8 * P], BF16, tag="qTp", name="qT_p")
        _copy(nc.vector, q16[:], q_nat[:])
        for i2 in range(4):
            pt = tpp.tile([P, 256], BF16, tag="tp", name="pt")
            for k in range(2):
                i = 2 * i2 + k
                nc.tensor.transpose(pt[:, P * k:P * (k + 1)],
                                    q16[:, P * i:P * (i + 1)], identb[:])
            cp = nc.vector if i2 % 2 == 0 else nc.scalar
            _copy(cp, qT_p[:, 256 * i2:256 * (i2 + 1)], pt[:])

        # ---- qT projection ----
        qdst = qT[:].rearrange("p (i two b) -> p i two b", two=2, b=P)
        for tb in range(2):
            for x in range(2):
                pp = prp.tile([P, 512], F32, tag="pe", name="pp")
                nc.tensor.matmul(pp[:], wq2[tb][:],
                                 qT_p[:, 512 * x:512 * (x + 1)],
                                 start=True, stop=True)
                ppv = pp[:].rearrange("p (i b) -> p i b", b=P)
                cp = nc.vector if (tb + x) % 2 == 0 else nc.scalar
                _copy(cp, qdst[:, 4 * x:4 * x + 4, tb, :], ppv)

        # ---- Wk^T via one PE transpose, then rq[parity] = Wk q'^T ----
        # (folds the k-projection into the q side: 8 matmuls instead of 32)
        ptw = prp.tile([P, 512], F32, tag="pe", name="ptw")
        nc.tensor.transpose(ptw[0:C, 0:C], wsts[1][0:C, 0:C],
                            identf[0:C, 0:C])
        nc.vector.tensor_copy(out=wkt[0][0:C, 0:C], in_=ptw[0:C, 0:C])
        nc.scalar.copy(out=wkt[1][0:C, C:P], in_=ptw[0:C, 0:C])
        for tb in range(2):
            for x in range(4):
                pp = prp.tile([P, 512], F32, tag="pe", name="pp")
                nc.tensor.matmul(pp[:], wkt[tb][:],
                                 qT[:, 512 * x:512 * (x + 1)],
                                 start=True, stop=True)
                cp = nc.vector if (tb + x) % 2 == 0 else nc.scalar
                _copy(cp, rq[tb][:, 512 * x:512 * (x + 1)], pp[:])

        # ---- v: natural layout chunks + ones column ----
        for g in range(8):
            ones_dst = v_tiles[g][:].rearrange(
                "p (k s) -> p k s", s=65)[:, :, C:C + 1]
            nc.vector.tensor_copy(out=ones_dst, in_=ones16[:, :, None])
        # one psum tile per parity: matmuls from different row tile
        # positions must never write the same PSUM tile (hw gotcha)
        for g in range(8):
            for tb in range(2):
                vp = prp.tile([P, 512], F32, tag="po", name="vp")
                for j in range(8):
                    c_ = 16 * g + 2 * j + tb
                    s, jj = c_ // 8, (c_ % 8) // 2
                    nc.tensor.matmul(
                        vp[:, C * j:C * (j + 1)],
                        hT_tiles[s][:, P * jj:P * (jj + 1)],
                        wv2[tb][:],
                        start=True, stop=True)
                dv = v_tiles[g][:].rearrange("p (k two s) -> p k two s",
                                             two=2, s=65)
                vpv = vp[:].rearrange("p (k c) -> p k c", c=C)
                cp = nc.vector if tb == 0 else nc.scalar
                _copy(cp, dv[:, :, tb, 0:C], vpv)

    # ---------------- steady state ----------------
    scp = ctx_.enter_context(tc.tile_pool(name="sc_psum", bufs=SCB, space="PSUM"))
    scpb = ctx_.enter_context(tc.tile_pool(name="scb_psum", bufs=SCB, space="PSUM"))
    otp = ctx_.enter_context(tc.tile_pool(name="ot_psum", bufs=1, space="PSUM"))
    ptp = ctx_.enter_context(tc.tile_pool(name="pt_psum", bufs=2, space="PSUM"))
    # separate pools for the ACT- and DVE-written exp halves: a shared tile
    # would serialize the two engines via co-writer/co-reader ordering
    xpa = ctx_.enter_context(tc.tile_pool(name="expA", bufs=XB))
    xpb = ctx_.enter_context(tc.tile_pool(name="expB", bufs=XB))
    eps = ctx_.enter_context(tc.tile_pool(name="epi", bufs=4))

    for h in range(NPASS_RUN):
        q0 = QPASS * h
        outT = otp.tile([P, QPASS], F32, tag="outT")

        def s1(c, scab):
            # stationary = the parity-packed hidden^T block itself (chunks
            # 2j and 2j+1 share it); moving = rq[parity], whose zero half
            # cancels the other parity's rows exactly
            hs = hT_tiles[c // 8][:, P * ((c % 8) // 2):P * ((c % 8) // 2 + 1)]
            qm = rq[c % 2]
            for x in range(2):
                nc.tensor.matmul(
                    scab[x][:],
                    hs,
                    qm[:, q0 + 512 * x:q0 + 512 * (x + 1)],
                    start=True, stop=True)

        def sc_pair():
            a = scp.tile([P, 512], F32, tag="scA", name="scA")
            b = scpb.tile([P, 512], F32, tag="scB", name="scB")
            return (a, b)

        sc_cur = sc_pair()
        s1(0, sc_cur)
        for c in range(NKVC_RUN):
            # exp split across two engines so neither stalls the PE (which
            # must stream gap-free to stay at the warm 2.4 GHz clock):
            # ACT does the exact exp on cols [0:512), DVE emits the
            # Schraudolph bf16 bit-pattern on cols [512:1024).
            expA = xpa.tile([P, 512], BF16, tag="expA")
            expB = xpb.tile([P, 512], BF16, tag="expB")
            nc.scalar.activation(expA[:], sc_cur[0][:], AF.Exp, scale=0.125)
            nc.vector.tensor_scalar(
                out=expB[:].bitcast(I16),
                in0=sc_cur[1][:],
                scalar1=SCH_A, scalar2=SCH_B,
                op0=mybir.AluOpType.mult, op1=mybir.AluOpType.add)
            if c + 1 < NKVC_RUN:
                sc_nxt = sc_pair()
                s1(c + 1, sc_nxt)
                sc_cur = sc_nxt
            vt = v_tiles[c // 16][:, 65 * (c % 16):65 * (c % 16) + 65]
            for x, ex in enumerate((expA, expB)):
                nc.tensor.matmul(
                    outT[0:65, 512 * x:512 * (x + 1)],
                    vt,
                    ex[:],
                    start=(c == 0), stop=(c == NKVC_RUN - 1))

        # ---- epilogue for this pass: transpose blocks (packed 4 per PSUM
        # tile so they stream back-to-back instead of ring-stalling on the
        # muls), normalize by the ones-row sum (alternating DVE/ACT), one
        # contiguous out-DMA
        oT_sb = eps.tile([P, QPASS], F32, tag="oT_sb")
        nc.vector.tensor_copy(out=oT_sb[0:65, 0:512], in_=outT[0:65, 0:512])
        nc.scalar.copy(out=oT_sb[0:65, 512:1024], in_=outT[0:65, 512:1024])
        resbig = eps.tile([P, 512], F32, tag="resbig")
        pts = []
        for q4 in range(2):
            pt4 = ptp.tile([P, 4 * 65], F32, tag="pt")
            pts.append(pt4)
            for k in range(4):
                j2 = 4 * q4 + k
                nc.tensor.transpose(pt4[:, 65 * k:65 * k + 65],
                                    oT_sb[0:65, P * j2:P * (j2 + 1)],
                                    identf[0:65, 0:65])
        for j2 in range(8):
            pt = pts[j2 // 4][:, 65 * (j2 % 4):65 * (j2 % 4) + 65]
            rcp = eps.tile([P, 1], F32, tag="rcp")
            nc.vector.reciprocal(rcp[:], pt[:, C:C + 1])
            dst = resbig[:, C * j2:C * (j2 + 1)]
            if j2 % 2 == 0:
                nc.vector.tensor_scalar_mul(dst, pt[:, 0:C], rcp[:])
            else:
                nc.scalar.mul(dst, pt[:, 0:C], rcp[:])
        eng = nc.sync if h % 2 == 0 else nc.scalar
        eng.dma_start(out=out_flat[:, 512 * h:512 * (h + 1)], in_=resbig[:])


def build_program(repeat=1):
    from contextlib import ExitStack

    nc = bacc.Bacc("TRN2", target_bir_lowering=False, debug=False,
                   num_devices=NCORES)
    hidden_d = nc.dram_tensor("hidden", [NKV, C], F32, kind="ExternalInput").ap()
    query_d = nc.dram_tensor("query", [NQ, C], F32, kind="ExternalInput").ap()
    wq_d = nc.dram_tensor("Wq", [C, C], F32, kind="ExternalInput").ap()
    wk_d = nc.dram_tensor("Wk", [C, C], F32, kind="ExternalInput").ap()
    wv_d = nc.dram_tensor("Wv", [C, C], F32, kind="ExternalInput").ap()
    out_d = nc.dram_tensor("out", [NQ, C], F32, kind="ExternalOutput").ap()

    with tile.TileContext(nc) as tc:
        with ExitStack() as ctx_:
            if repeat == 1:
                _emit(nc, tc, ctx_, hidden_d, query_d, wq_d, wk_d, wv_d,
                      out_d)
            else:
                with tc.For_i(0, repeat, 1):
                    _emit(nc, tc, ctx_, hidden_d, query_d, wq_d, wk_d, wv_d,
                          out_d)
    nc.compile()
    return nc


_NC_CACHE = {}


def kernel(hidden, query, Wq, Wk, Wv):
    hidden = np.ascontiguousarray(np.asarray(hidden, dtype=np.float32))
    query = np.ascontiguousarray(np.asarray(query, dtype=np.float32))
    Wq = np.ascontiguousarray(np.asarray(Wq, dtype=np.float32))
    Wk = np.ascontiguousarray(np.asarray(Wk, dtype=np.float32))
    Wv = np.ascontiguousarray(np.asarray(Wv, dtype=np.float32))

    if "nc" not in _NC_CACHE:
        _NC_CACHE["nc"] = build_program()
    nc = _NC_CACHE["nc"]

    in_maps = [
        {"hidden": hidden, "query": query[i * NQ:(i + 1) * NQ],
         "Wq": Wq, "Wk": Wk, "Wv": Wv}
        for i in range(NCORES)
    ]
    res = run_bass_kernel_spmd(nc, in_maps, core_ids=list(range(NCORES)))
    return np.concatenate([res.results[i]["out"] for i in range(NCORES)],
                          axis=0)


# revision 41
# speedup vs baseline: 1.0275x; 1.0115x over previous
"""Trainium2 Bass kernel for sparse-attention (full 16384x16384 attention,
64-dim head, 64x64 projections).

Sharding: query rows split across 8 cores (2048 rows each); hidden (K/V
source) and the 64x64 projection weights are replicated on every core.

Per-core algorithm:
  prologue (all-bf16, all matmuls use full-128-row stationaries so the PE
  HAM clock gate sees a busy array and runs at 2.4 GHz):
    warmup   ~16 dummy matmuls gated on the first h16 tile: un-throttle the
             PE HAM clock gate right before the real matmuls start
    h16      hidden loaded f32 (DMAs split across the SP and ACT dispatch
             queues), converted to bf16 (DVE-heavy)
    hT       PE-transposed 128x128 blocks of h16; 16 packed tiles [128,512],
             top half = even kv chunks' hidden^T, bottom = odd chunks'.
             These double as the stage-1 stationaries: no k is ever
             materialized, because S^T = H Wk Wq^T Q^T = H @ (Wk q'^T).
    qT,rq    q'^T = Wq^T Q^T [128, 2048] (rows 64:128 exact zeros), then
             rq[parity] = Wk q'^T with the 64 data rows placed at
             partitions 0:64 (even kv chunks) or 64:128 (odd) and exact
             zeros elsewhere, so contraction with the parity-packed hT
             stationary picks out exactly one chunk (K=128 keeps the HAM
             clock gate warm).
    v        v in natural [kv, c] chunks, each padded with a ones column
             (the ones column makes stage 2 accumulate the softmax
             denominator as row 64 of outT)
  steady state, per q-pass (1024 q cols) x 128 kv chunks, software-
  pipelined so the PE streams gap-free at 2.4 GHz (863 ns/chunk):
    stage1: scA[kv=128,512], scB[kv=128,512] = hT_blk^T @ rq  (PE->PSUM)
    exp:    expA = exp(scA/8) on ACT; expB = Schraudolph bf16 bit-pattern
            round(A*scB+B) as int16 on DVE.  Two engines in parallel, each
            under the PE's 864 ns/chunk budget; fully separate tiles so the
            tile framework cannot serialize them.
    stage2: outT[65, q] += v_chunk_aug^T @ exp{A,B}            (PE, PSUM)
  epilogue per pass: outT -> SBUF (copy split ACT/DVE), PE-transpose
  128-col blocks, multiply by reciprocal of the ones-column sum
  (alternating DVE/ACT), DMA out.

kv ordering is permuted (chunk c holds kv rows {128p + c}) - harmless since
softmax fully reduces over kv and k/v use the same permutation.  q ordering:
column g = 128t + p corresponds to query row 16p + t of this core's shard.
"""

import numpy as np

import concourse.bass as bass
import concourse.mybir as mybir
import concourse.tile as tile
from concourse import bacc
from concourse.bass_utils import run_bass_kernel_spmd
from concourse.masks import make_identity

F32 = mybir.dt.float32
BF16 = mybir.dt.bfloat16
I16 = mybir.dt.int16
AF = mybir.ActivationFunctionType

# Schraudolph exp for bf16: bitpattern(exp(s/8)) ~= round(A*s + B).
# A = 2^7 * log2(e) / 8 (folds the 1/sqrt(64) score scale); B centers the
# exponent at 127 with c=0.04368 minimizing max relative error (~3%).
SCH_A = 128.0 * 0.125 * 1.4426950408889634
SCH_B = 128.0 * (127.0 - 0.04368)

NKV = 16384
C = 64
NCORES = 8
NQ = NKV // NCORES  # 2048 q rows per core
P = 128

NKVC = NKV // P      # 128 kv chunks
NQC = NQ // P        # 16 q chunks
QPASS = 1024         # q columns processed per pass
NPASS = NQ // QPASS  # 2

import os
NKVC_RUN = int(os.environ.get("NKVC_RUN", NKVC))
NPASS_RUN = int(os.environ.get("NPASS_RUN", NPASS))
SCB = int(os.environ.get("SCB", "2"))
XB = int(os.environ.get("XB", "4"))
WARMN = int(os.environ.get("WARMN", "16"))


def _copy(eng, out, in_):
    if hasattr(eng, "tensor_copy"):
        eng.tensor_copy(out=out, in_=in_)
    else:
        eng.copy(out=out, in_=in_)


def _emit(nc, tc, ctx_, hidden_d, query_d, wq_d, wk_d, wv_d, out_d):
    from contextlib import ExitStack

    # ---------------- constants ----------------
    consts = ctx_.enter_context(tc.tile_pool(name="consts", bufs=1))
    # top/bot variants: w*t = [[W],[0]], w*b = [[0],[W]] so projections from
    # the parity-packed hT tiles contract over all 128 partitions (the zero
    # half exactly cancels the other parity's rows)
    wq2 = [consts.tile([P, P], BF16, name=f"wq2{i}") for i in range(2)]
    # Wk is folded into the q side: S^T = H (Wk q'^T), so stage 1 uses the
    # hT tiles directly as stationary and no kT is ever materialized.
    # wkt[0] places Wk^T at out-partitions 0:64 (for even kv chunks),
    # wkt[1] at 64:128 (odd chunks); all other entries zero.
    wkt = [consts.tile([P, P], BF16, name=f"wkt{i}") for i in range(2)]
    wv2 = [consts.tile([P, C], BF16, name=f"wv2{i}") for i in range(2)]
    # one staging tile per weight: a shared one serializes DMA->copy->DMA
    wsts = [consts.tile([P, 2 * P], F32, name=f"wst{i}") for i in range(3)]
    ones16 = consts.tile([P, 16], BF16, name="ones16")
    identb = consts.tile([P, P], BF16, name="identb")
    identf = consts.tile([P, P], F32, name="identf")
    warmw = consts.tile([P, P], BF16, name="warmw")
    warmm = consts.tile([P, 512], BF16, name="warmm")

    nc.vector.memset(warmw[:], 0.0)
    nc.vector.memset(warmm[:], 0.0)
    nc.vector.memset(ones16[:], 1.0)
    nc.vector.memset(wkt[0][:], 0.0)
    nc.vector.memset(wkt[1][:], 0.0)
    make_identity(nc, identb)
    make_identity(nc, identf)

    # persistent operand tiles (hT doubles as the stage-1 stationary)
    big = ctx_.enter_context(tc.tile_pool(name="big", bufs=1))
    hT_tiles = [big.tile([P, 4 * P], BF16, tag=f"hT{s}", name=f"hT{s}")
                for s in range(16)]
    qT = big.tile([P, NQ], BF16, tag="qt", name="qT")
    # rq[parity] = (Wk q'^T) with the data on partitions 0:64 (even kv
    # chunks) or 64:128 (odd); the other half is exact zeros so contraction
    # with the parity-packed hT stationary picks out one chunk.
    rq = [big.tile([P, NQ], BF16, tag=f"rq{i}", name=f"rq{i}")
          for i in range(2)]
    v_tiles = [big.tile([P, 16 * 65], BF16, tag=f"v{i}", name=f"v{i}")
               for i in range(8)]

    hidden_r = hidden_d.rearrange("(p t) c -> p (t c)", p=P)   # [128, 8192]
    query_r = query_d.rearrange("(p t) c -> p (t c)", p=P)     # [128, 1024]
    out_flat = out_d.rearrange("(p t) c -> p (t c)", p=P)      # [128, 1024]

    with ExitStack() as pro:
        hp = pro.enter_context(tc.tile_pool(name="hnat", bufs=1))
        htp = pro.enter_context(tc.tile_pool(name="ht", bufs=1))
        tpp = pro.enter_context(
            tc.tile_pool(name="tp_psum", bufs=2, space="PSUM"))
        prp = pro.enter_context(
            tc.tile_pool(name="proj_psum", bufs=2, space="PSUM"))
        wpp = pro.enter_context(
            tc.tile_pool(name="warm_psum", bufs=2, space="PSUM"))

        # ---- weight DMAs first: tiny (16KB), and they gate the projection
        # chain; separate stagings avoid DMA->copy->DMA serialization
        for i, w_d in enumerate((wq_d, wk_d, wv_d)):
            nc.vector.memset(wsts[i][:], 0.0)
            nc.sync.dma_start(out=wsts[i][0:C, 0:C], in_=w_d[:, :])
            if i != 1:  # Wk is only needed once (transposed on the PE)
                nc.scalar.dma_start(out=wsts[i][C:P, P:P + C], in_=w_d[:, :])

        # ---- hidden + query loads, split across SP/ACT dispatch queues
        h_tiles = [hp.tile([P, 8 * P], F32, tag=f"h{g}", name=f"h{g}")
                   for g in range(8)]
        h16_tiles = [hp.tile([P, 8 * P], BF16, tag=f"h16_{g}", name=f"h16_{g}")
                     for g in range(8)]
        for g in range(8):
            eng = nc.sync if g % 2 == 0 else nc.scalar
            eng.dma_start(out=h_tiles[g][:],
                          in_=hidden_r[:, 1024 * g:1024 * (g + 1)])
        q_nat = hp.tile([P, NQC * C], F32, tag="qnat", name="q_nat")
        q16 = hp.tile([P, NQC * C], BF16, tag="q16", name="q16")
        nc.sync.dma_start(out=q_nat[:], in_=query_r[:, :])

        # ---- weight copies (DVE, early: they gate the projections)
        for i, wpair in ((0, wq2), (2, wv2)):
            wcols = wpair[0].shape[1]
            nc.vector.tensor_copy(out=wpair[0][:], in_=wsts[i][:, 0:wcols])
            nc.vector.tensor_copy(out=wpair[1][:],
                                  in_=wsts[i][:, P:P + wcols])

        # ---- convert hidden to bf16 (DVE-heavy: the ACT copy path is 2x
        # slower and the conversion tail gates the transposes)
        for g in range(8):
            cp = nc.scalar if g in (5, 7) else nc.vector
            _copy(cp, h16_tiles[g][:], h_tiles[g][:])

        # ---- PE warmup: dummy matmuls gated on the first h16 tile, so the
        # ~3.4us HAM un-throttle window completes right as the transposes
        # start (warming earlier is wasted - the gate re-throttles after
        # ~3.4us of idle)
        for i in range(WARMN):
            wt = wpp.tile([P, 512], F32, tag="warm", name="warm")
            nc.tensor.matmul(wt[:], warmw[:],
                             h16_tiles[0][:, 0:512], start=True, stop=True)
        for i4 in range(16):  # quads of 128-col transposes -> one copy each
            pt = tpp.tile([P, 512], BF16, tag="tp", name="pt")
            for k in range(4):
                i = 4 * i4 + k
                g, b = i // 8, i % 8
                nc.tensor.transpose(pt[:, P * k:P * (k + 1)],
                                    h16_tiles[g][:, P * b:P * (b + 1)],
                                    identb[:])
            cp = nc.vector if i4 % 2 == 0 else nc.scalar
            _copy(cp, hT_tiles[i4][:], pt[:])

        # ---- q: convert + transpose (packed like hT) ----
        qT_p = htp.tile([P, 8 * P], BF16, tag="qTp", name="qT_p")
        _copy(nc.vector, q16[:], q_nat[:])
        for i2 in range(4):
            pt = tpp.tile([P, 256], BF16, tag="tp", name="pt")
            for k in range(2):
                i = 2 * i2 + k
                nc.tensor.transpose(pt[:, P * k:P * (k + 1)],
                                    q16[:, P * i:P * (i + 1)], identb[:])
            cp = nc.vector if i2 % 2 == 0 else nc.scalar
            _copy(cp, qT_p[:, 256 * i2:256 * (i2 + 1)], pt[:])

        # ---- qT projection ----
        qdst = qT[:].rearrange("p (i two b) -> p i two b", two=2, b=P)
        for tb in range(2):
            for x in range(2):
                pp = prp.tile([P, 512], F32, tag="pe", name="pp")
                nc.tensor.matmul(pp[:], wq2[tb][:],
                                 qT_p[:, 512 * x:512 * (x + 1)],
                                 start=True, stop=True)
                ppv = pp[:].rearrange("p (i b) -> p i b", b=P)
                cp = nc.vector if (tb + x) % 2 == 0 else nc.scalar
                _copy(cp, qdst[:, 4 * x:4 * x + 4, tb, :], ppv)

        # ---- Wk^T via one PE transpose, then rq[parity] = Wk q'^T ----
        # (folds the k-projection into the q side: 8 matmuls instead of 32)
        ptw = prp.tile([P, 512], F32, tag="pe", name="ptw")
        nc.tensor.transpose(ptw[0:C, 0:C], wsts[1][0:C, 0:C],
                            identf[0:C, 0:C])
        nc.vector.tensor_copy(out=wkt[0][0:C, 0:C], in_=ptw[0:C, 0:C])
        nc.scalar.copy(out=wkt[1][0:C, C:P], in_=ptw[0:C, 0:C])
        for tb in range(2):
            for x in range(4):
                pp = prp.tile([P, 512], F32, tag="pe", name="pp")
                nc.tensor.matmul(pp[:], wkt[tb][:],
                                 qT[:, 512 * x:512 * (x + 1)],
                                 start=True, stop=True)
                cp = nc.vector if (tb + x) % 2 == 0 else nc.scalar
                _copy(cp, rq[tb][:, 512 * x:512 * (x + 1)], pp[:])

        # ---- v: natural layout chunks + ones column ----
        for g in range(8):
            ones_dst = v_tiles[g][:].rearrange(
                "p (k s) -> p k s", s=65)[:, :, C:C + 1]
            nc.vector.tensor_copy(out=ones_dst, in_=ones16[:, :, None])
        # one psum tile per parity: matmuls from different row tile
        # positions must never write the same PSUM tile (hw gotcha)
        for g in range(8):
            for tb in range(2):
                vp = prp.tile([P, 512], F32, tag="po", name="vp")
                for j in range(8):
                    c_ = 16 * g + 2 * j + tb
                    s, jj = c_ // 8, (c_ % 8) // 2
                    nc.tensor.matmul(
                        vp[:, C * j:C * (j + 1)],
                        hT_tiles[s][:, P * jj:P * (jj + 1)],
                        wv2[tb][:],
                        start=True, stop=True)
                dv = v_tiles[g][:].rearrange("p (k two s) -> p k two s",
                                             two=2, s=65)
                vpv = vp[:].rearrange("p (k c) -> p k c", c=C)
                cp = nc.vector if tb == 0 else nc.scalar
                _copy(cp, dv[:, :, tb, 0:C], vpv)

    # ---------------- steady state ----------------
    scp = ctx_.enter_context(tc.tile_pool(name="sc_psum", bufs=SCB, space="PSUM"))
    scpb = ctx_.enter_context(tc.tile_pool(name="scb_psum", bufs=SCB, space="PSUM"))
    otp = ctx_.enter_context(tc.tile_pool(name="ot_psum", bufs=1, space="PSUM"))
    ptp = ctx_.enter_context(tc.tile_pool(name="pt_psum", bufs=2, space="PSUM"))
    # separate pools for the ACT- and DVE-written exp halves: a shared tile
    # would serialize the two engines via co-writer/co-reader ordering
    xpa = ctx_.enter_context(tc.tile_pool(name="expA", bufs=XB))
    xpb = ctx_.enter_context(tc.tile_pool(name="expB", bufs=XB))
    eps = ctx_.enter_context(tc.tile_pool(name="epi", bufs=4))

    for h in range(NPASS_RUN):
        q0 = QPASS * h
        outT = otp.tile([P, QPASS], F32, tag="outT")

        def s1(c, scab):
            # stationary = the parity-packed hidden^T block itself (chunks
            # 2j and 2j+1 share it); moving = rq[parity], whose zero half
            # cancels the other parity's rows exactly
            hs = hT_tiles[c // 8][:, P * ((c % 8) // 2):P * ((c % 8) // 2 + 1)]
            qm = rq[c % 2]
            for x in range(2):
                nc.tensor.matmul(
                    scab[x][:],
                    hs,
                    qm[:, q0 + 512 * x:q0 + 512 * (x + 1)],
                    start=True, stop=True)

        def sc_pair():
            a = scp.tile([P, 512], F32, tag="scA", name="scA")
            b = scpb.tile([P, 512], F32, tag="scB", name="scB")
            return (a, b)

        sc_cur = sc_pair()
        s1(0, sc_cur)
        for c in range(NKVC_RUN):
            # exp split across two engines so neither stalls the PE (which
            # must stream gap-free to stay at the warm 2.4 GHz clock):
            # ACT does the exact exp on cols [0:512), DVE emits the
            # Schraudolph bf16 bit-pattern on cols [512:1024).
            sc_prev = sc_cur
            if c + 1 < NKVC_RUN:
                sc_nxt = sc_pair()
                s1(c + 1, sc_nxt)
                sc_cur = sc_nxt
            expA = xpa.tile([P, 512], BF16, tag="expA")
            expB = xpb.tile([P, 512], BF16, tag="expB")
            nc.scalar.activation(expA[:], sc_prev[0][:], AF.Exp, scale=0.125)
            nc.vector.tensor_scalar(
                out=expB[:].bitcast(I16),
                in0=sc_prev[1][:],
                scalar1=SCH_A, scalar2=SCH_B,
                op0=mybir.AluOpType.mult, op1=mybir.AluOpType.add)
            vt = v_tiles[c // 16][:, 65 * (c % 16):65 * (c % 16) + 65]
            for x, ex in enumerate((expA, expB)):
                nc.tensor.matmul(
                    outT[0:65, 512 * x:512 * (x + 1)],
                    vt,
                    ex[:],
                    start=(c == 0), stop=(c == NKVC_RUN - 1))

        # ---- epilogue for this pass: transpose blocks (packed 4 per PSUM
        # tile so they stream back-to-back instead of ring-stalling on the
        # muls), normalize by the ones-row sum (alternating DVE/ACT), one
        # contiguous out-DMA
        oT_sb = eps.tile([P, QPASS], F32, tag="oT_sb")
        nc.vector.tensor_copy(out=oT_sb[0:65, 0:512], in_=outT[0:65, 0:512])
        nc.scalar.copy(out=oT_sb[0:65, 512:1024], in_=outT[0:65, 512:1024])
        resbig = eps.tile([P, 512], F32, tag="resbig")
        pts = []
        for q4 in range(2):
            pt4 = ptp.tile([P, 4 * 65], F32, tag="pt")
            pts.append(pt4)
            for k in range(4):
                j2 = 4 * q4 + k
                nc.tensor.transpose(pt4[:, 65 * k:65 * k + 65],
                                    oT_sb[0:65, P * j2:P * (j2 + 1)],
                                    identf[0:65, 0:65])
        for j2 in range(8):
            pt = pts[j2 // 4][:, 65 * (j2 % 4):65 * (j2 % 4) + 65]
            rcp = eps.tile([P, 1], F32, tag="rcp")
            nc.vector.reciprocal(rcp[:], pt[:, C:C + 1])
            dst = resbig[:, C * j2:C * (j2 + 1)]
            if j2 % 2 == 0:
                nc.vector.tensor_scalar_mul(dst, pt[:, 0:C], rcp[:])
            else:
                nc.scalar.mul(dst, pt[:, 0:C], rcp[:])
        eng = nc.sync if h % 2 == 0 else nc.scalar
        eng.dma_start(out=out_flat[:, 512 * h:512 * (h + 1)], in_=resbig[:])


def build_program(repeat=1):
    from contextlib import ExitStack

    nc = bacc.Bacc("TRN2", target_bir_lowering=False, debug=False,
                   num_devices=NCORES)
    hidden_d = nc.dram_tensor("hidden", [NKV, C], F32, kind="ExternalInput").ap()
    query_d = nc.dram_tensor("query", [NQ, C], F32, kind="ExternalInput").ap()
    wq_d = nc.dram_tensor("Wq", [C, C], F32, kind="ExternalInput").ap()
    wk_d = nc.dram_tensor("Wk", [C, C], F32, kind="ExternalInput").ap()
    wv_d = nc.dram_tensor("Wv", [C, C], F32, kind="ExternalInput").ap()
    out_d = nc.dram_tensor("out", [NQ, C], F32, kind="ExternalOutput").ap()

    with tile.TileContext(nc) as tc:
        with ExitStack() as ctx_:
            if repeat == 1:
                _emit(nc, tc, ctx_, hidden_d, query_d, wq_d, wk_d, wv_d,
                      out_d)
            else:
                with tc.For_i(0, repeat, 1):
                    _emit(nc, tc, ctx_, hidden_d, query_d, wq_d, wk_d, wv_d,
                          out_d)
    nc.compile()
    return nc


_NC_CACHE = {}


def kernel(hidden, query, Wq, Wk, Wv):
    hidden = np.ascontiguousarray(np.asarray(hidden, dtype=np.float32))
    query = np.ascontiguousarray(np.asarray(query, dtype=np.float32))
    Wq = np.ascontiguousarray(np.asarray(Wq, dtype=np.float32))
    Wk = np.ascontiguousarray(np.asarray(Wk, dtype=np.float32))
    Wv = np.ascontiguousarray(np.asarray(Wv, dtype=np.float32))

    if "nc" not in _NC_CACHE:
        _NC_CACHE["nc"] = build_program()
    nc = _NC_CACHE["nc"]

    in_maps = [
        {"hidden": hidden, "query": query[i * NQ:(i + 1) * NQ],
         "Wq": Wq, "Wk": Wk, "Wv": Wv}
        for i in range(NCORES)
    ]
    res = run_bass_kernel_spmd(nc, in_maps, core_ids=list(range(NCORES)))
    return np.concatenate([res.results[i]["out"] for i in range(NCORES)],
                          axis=0)
